# revision 1
# baseline (speedup 1.0000x reference)
"""Trainium2 Bass kernel for per-position multi-head "attention across heads".

Reference math (per position r):
    Q = x @ Wq.T ; K = x @ Wk.T ; V = x @ Wv.T          (H=1024, nh=16, hd=64)
    scores[r, i, j] = (1/8) * sum_d Q[r,i,d] * K[r,j,d]   -> [nh, nh] per position
    attn = softmax(scores, axis=-1)
    out[r, i, :] = sum_j attn[r,i,j] * V[r, j, :]

Strategy (8 NeuronCores, data-parallel over the 8192 = B*L positions):
  - Each core handles R=1024 positions: x_shard [1024, 1024] plus full Wq/Wk/Wv.
  - bf16 compute on the TensorEngine (PSUM accumulation in fp32).
  - x and W are cast to bf16 in DRAM (SWDGE cast DMA), then x^T / W^T SBUF
    tiles are produced by XBAR DMA-transpose loads (no PE transposes).
  - Projections compute NATURAL-layout Q/K/V tiles [r, o], evicted densely
    (DVE, cast bf16) and stored to a DRAM staging buffer laid out
    [r, head, 128] (d in cols 0..63, cols 64..127 untouched padding).
  - One DMA-transpose per matrix then yields the "position-major" operand
    pm[d (partitions 0..63), r*16 + head] used by the attention phase
    (partitions 64..127 hold padding garbage and are never read).
  - Scores for 8 positions at a time via ONE K=64 matmul:
      lhsT = K^T pm slice [64, 128], rhs = Q^T pm slice [64, 128]
      -> PSUM [ (pos,j), (pos,i) ] with garbage off-diagonal blocks.
  - exp via ScalarE (no max subtraction needed: |scores| <= ~3), mask off-diag
    garbage blocks with a precomputed block mask, then
  - AV via matmul with contraction over (pos, j): lhsT = V_stack [ (pos,j), d ]
    (built by a PE transpose of the V pm slice), rhs = masked exp. A second
    1-row matmul with a ones lhsT accumulates softmax denominators into
    PSUM row 64.
  - PE-transpose the [65, 128] result back to [ (pos,i), d|Z ] layout, then
    normalize rows by 1/Z on the VectorE and DMA straight to HBM.
"""

import numpy as np

import concourse.bass as bass
import concourse.mybir as mybir
import concourse.tile as tile
from concourse import bacc

F32 = mybir.dt.float32
BF16 = mybir.dt.bfloat16

B, L, H = 4, 2048, 1024
NH, HD = 16, 64
P = 128
N_CORES = 8
R = (B * L) // N_CORES          # positions per core = 1024
KC = H // P                     # contraction chunks = 8
OC = H // P                     # output-feature chunks = 8
GS = 8                          # positions per attention group
GB = 4                          # groups per PSUM-bank batch
SCALE = 1.0 / np.sqrt(HD)


def build_nc(r_core=R):
    RC = r_core
    RT = RC // P                # x row tiles
    NGRP = RC // GS             # attention groups
    NBATCH = NGRP // GB         # group batches

    nc = bacc.Bacc(None, target_bir_lowering=False, debug=False)

    x = nc.dram_tensor("x", [RC, H], F32, kind="ExternalInput")
    Ws = {m: nc.dram_tensor(f"W{m}", [H, H], F32, kind="ExternalInput")
          for m in ("q", "k", "v")}
    ident_bf_d = nc.dram_tensor("ident_bf", [P, P], BF16, kind="ExternalInput")
    ident_f32_d = nc.dram_tensor("ident_f32", [P, P], F32, kind="ExternalInput")
    blkmask_d = nc.dram_tensor("blkmask", [P, P], BF16, kind="ExternalInput")
    ones_col_d = nc.dram_tensor("ones_col", [P, 1], BF16, kind="ExternalInput")
    out = nc.dram_tensor("out", [RC, H], F32, kind="ExternalOutput")

    with tile.TileContext(nc) as tc:
        with tc.tile_pool(name="const", bufs=1) as constp, \
             tc.tile_pool(name="persist", bufs=1) as persist, \
             tc.tile_pool(name="dram", bufs=1, space="DRAM") as dram:
            ident_bf = constp.tile([P, P], BF16)
            ident_f32 = constp.tile([P, P], F32)
            blkmask = constp.tile([P, P], BF16)
            ones_col = constp.tile([P, 1], BF16)
            nc.sync.dma_start(ident_bf[:], ident_bf_d[:])
            nc.sync.dma_start(ident_f32[:], ident_f32_d[:])
            nc.sync.dma_start(blkmask[:], blkmask_d[:])
            nc.sync.dma_start(ones_col[:], ones_col_d[:])

            # DRAM staging (bf16)
            xbf = [dram.tile([RC, H // 2], BF16, name=f"xbf{h}")
                   for h in range(2)]
            # W bf16 staging split into separate k-half tensors: Tile
            # tracks DRAM deps per-tensor, so first-half transpose-loads
            # unblock after half the cast traffic.
            wbf = {m: [dram.tile([H, H // 2], BF16, name=f"wbf_{m}{h}")
                       for h in range(2)]
                   for m in ("q", "k", "v")}
            stag = {m: dram.tile([RC, NH, P], BF16, name=f"stag_{m}")
                    for m in ("q", "k", "v")}
            ostg = dram.tile([RC // (GB * GS), P, GB, HD], F32)

            # persistent SBUF tensors
            xT = persist.tile([P, KC, RC], BF16)        # x^T chunks [k, kc, r]
            # position-major Q^T/K^T/V^T: pm[d, r*NH + head] on partitions
            # 0..63; partitions 64..127 are transpose padding (never read).
            pm = {m: persist.tile([P, RC * NH], BF16, name=f"pm_{m}")
                  for m in ("q", "k", "v")}

            # ---- phase 0: bf16 casts in DRAM + transposed loads ----
            # x whole (it gates everything), then W k-halves with all
            # first halves before second halves: the K-accumulation
            # consumes k-chunks in order, so projections start after
            # roughly half the cast traffic.
            HH = H // 2
            for hh in range(2):
                nc.gpsimd.dma_start(xbf[hh][:], x[:, hh * HH:(hh + 1) * HH])
                for m in ("q", "k", "v"):
                    nc.gpsimd.dma_start(
                        wbf[m][hh][:], Ws[m][:, hh * HH:(hh + 1) * HH])

            # ---- phase 1: projections (natural layout) -> DRAM staging ----
            with tc.tile_pool(name="wT", bufs=1) as wTp, \
                 tc.tile_pool(name="qnat", bufs=3) as qnatp, \
                 tc.tile_pool(name="projps", bufs=6, space="PSUM") as projpsp:
                wT = {}
                for m in ("q", "k", "v"):
                    wT[m] = wTp.tile([P, KC, H], BF16, tag=f"wT_{m}",
                                     name=f"wT_{m}")
                KH = KC // 2
                for kc in range(KC):
                    csl = slice((kc % KH) * P, (kc % KH + 1) * P)
                    nc.sync.dma_start_transpose(
                        xT[:, kc, :], xbf[kc // KH][:, csl])
                    for m in ("q", "k", "v"):
                        nc.sync.dma_start_transpose(
                            wT[m][:, kc, :], wbf[m][kc // KH][:, csl])
                HF = RC // 2
                for rt in range(RT):
                    for m in ("q", "k", "v"):
                        # full-width staging tile: cols 64..127 are padding
                        # (memset keeps them finite; they land on pm
                        # partitions 64..127 which are never read)
                        qn = qnatp.tile([P, NH, P], BF16, tag="qn")
                        nc.vector.memset(qn[:], 0.0)
                        for oh in range(2):
                            pp = projpsp.tile([P, 512], F32)
                            for kc in range(KC):
                                nc.tensor.matmul(
                                    pp[:],
                                    xT[:, kc, rt * P:(rt + 1) * P],
                                    wT[m][:, kc, oh * 512:(oh + 1) * 512],
                                    start=(kc == 0), stop=(kc == KC - 1))
                            nc.vector.tensor_copy(
                                qn[:, oh * 8:(oh + 1) * 8, 0:HD],
                                pp[:].rearrange("p (i d) -> p i d", d=HD))
                        # contiguous store: rows rt*128.., all heads+pad
                        nc.gpsimd.dma_start(
                            stag[m][rt * P:(rt + 1) * P, :, :], qn[:])
                        # XBAR transpose quarters [ (r,i), 128 ] ->
                        # [128, (r,i)] as soon as each staging quarter is
                        # complete, so attention starts earlier.
                        if RT >= 4 and (rt + 1) % (RT // 4) == 0:
                            qf = (rt + 1) // (RT // 4) - 1
                            QF = RC // 4
                            nc.sync.dma_start_transpose(
                                pm[m][:, qf * QF * NH:(qf + 1) * QF * NH],
                                stag[m][qf * QF:(qf + 1) * QF]
                                .rearrange("r i d -> (r i) d"))
                        elif 1 < RT < 4 and rt in (RT // 2 - 1, RT - 1):
                            hf = 0 if rt == RT // 2 - 1 else 1
                            nc.sync.dma_start_transpose(
                                pm[m][:, hf * HF * NH:(hf + 1) * HF * NH],
                                stag[m][hf * HF:(hf + 1) * HF]
                                .rearrange("r i d -> (r i) d"))
                        elif RT == 1 and True:
                            pass
                if RT == 1:
                    for m in ("q", "k", "v"):
                        nc.sync.dma_start_transpose(
                            pm[m][:], stag[m].rearrange("r i d -> (r i) d"))

            # ---- phase 2: attention ----
            with tc.tile_pool(name="sps", bufs=2, space="PSUM") as spsp, \
                 tc.tile_pool(name="vps", bufs=2, space="PSUM") as vpsp, \
                 tc.tile_pool(name="avps", bufs=2, space="PSUM") as avpsp, \
                 tc.tile_pool(name="tps", bufs=2, space="PSUM") as tpsp, \
                 tc.tile_pool(name="att", bufs=5) as attp:
                for b in range(NBATCH):
                    ps = spsp.tile([P, GB, P], F32)
                    pv = vpsp.tile([P, GB, HD], BF16)
                    for g4 in range(GB):
                        r0 = (b * GB + g4) * GS
                        kap = pm["k"][0:HD, r0 * NH:(r0 + GS) * NH]
                        qap = pm["q"][0:HD, r0 * NH:(r0 + GS) * NH]
                        vap = pm["v"][0:HD, r0 * NH:(r0 + GS) * NH]
                        nc.tensor.matmul(
                            ps[:, g4, :], kap, qap,
                            start=(g4 == 0), stop=(g4 == GB - 1))
                        nc.tensor.matmul(
                            pv[:, g4, :], vap, ident_bf[0:HD, 0:HD],
                            is_transpose=True,
                            start=(g4 == 0), stop=(g4 == GB - 1))
                    E = attp.tile([P, GB, P], BF16, tag="E")
                    nc.scalar.activation(
                        E[:], ps[:], mybir.ActivationFunctionType.Exp,
                        scale=float(SCALE))
                    Em = attp.tile([P, GB, P], BF16, tag="Em")
                    nc.vector.tensor_tensor(
                        Em[:], E[:],
                        blkmask[:, None, :].to_broadcast((P, GB, P)),
                        mybir.AluOpType.mult)
                    # Vs gets a 65th column of ones: the AV matmul then
                    # emits the softmax denominator as PSUM row 64.
                    Vs = attp.tile([P, GB, HD + 1], BF16, tag="Vs")
                    nc.vector.tensor_copy(Vs[:, :, 0:HD], pv[:])
                    nc.vector.memset(Vs[:, :, HD], 1.0)
                    pav = avpsp.tile([65, GB, P], F32)
                    for g4 in range(GB):
                        nc.tensor.matmul(
                            pav[:, g4, :], Vs[:, g4, :], Em[:, g4, :],
                            start=(g4 == 0), stop=(g4 == GB - 1))
                    av = attp.tile([65, GB, P], F32, tag="av")
                    nc.vector.tensor_copy(av[:], pav[:])
                    pt = tpsp.tile([P, GB, 65], F32)
                    for g4 in range(GB):
                        nc.tensor.matmul(
                            pt[:, g4, :], av[:, g4, :], ident_f32[0:65, 0:65],
                            is_transpose=True,
                            start=(g4 == 0), stop=(g4 == GB - 1))
                    o_sb = attp.tile([P, GB, 65], F32, tag="o_sb")
                    nc.scalar.copy(o_sb[:], pt[:])
                    rz = attp.tile([P, GB], F32, tag="rz")
                    nc.vector.reciprocal(rz[:], o_sb[:, :, 64])
                    o_nrm = attp.tile([P, GB, HD], F32, tag="o_nrm")
                    nc.vector.tensor_tensor(
                        o_nrm[:], o_sb[:, :, 0:HD],
                        rz[:, :, None].to_broadcast((P, GB, HD)),
                        mybir.AluOpType.mult)
                    # store batch result contiguously to DRAM staging, then
                    # DRAM->DRAM rearrange [(s,i), g, d] -> rows b*32+g*8+s
                    # (SWDGE ring; keeps the sync HWDGE ring free)
                    nc.gpsimd.dma_start(ostg[b, :, :, :], o_nrm[:])
                    nc.gpsimd.dma_start(
                        out[b * GB * GS:(b + 1) * GB * GS, :]
                        .rearrange("(g s) (i d) -> s i g d", s=GS, d=HD),
                        ostg[b].rearrange("(s i) g d -> s i g d", s=GS))

    nc.compile()
    return nc


def _consts():
    import ml_dtypes
    ident = np.eye(P)
    blk = np.kron(np.eye(GS), np.ones((NH, NH)))
    return {
        "ident_bf": ident.astype(ml_dtypes.bfloat16),
        "ident_f32": ident.astype(np.float32),
        "blkmask": blk.astype(ml_dtypes.bfloat16),
        "ones_col": np.ones((P, 1), dtype=ml_dtypes.bfloat16),
    }


_NC_CACHE = {}


def kernel(x, Wq, Wk, Wv):
    from concourse.bass_utils import run_bass_kernel_spmd

    x = np.ascontiguousarray(np.asarray(x, dtype=np.float32))
    xf = x.reshape(B * L, H)
    consts = _consts()
    Wd = {"Wq": np.asarray(Wq, np.float32), "Wk": np.asarray(Wk, np.float32),
          "Wv": np.asarray(Wv, np.float32)}
    in_maps = []
    for c in range(N_CORES):
        m = {"x": np.ascontiguousarray(xf[c * R:(c + 1) * R]),
             "Wq": Wd["Wq"], "Wk": Wd["Wk"], "Wv": Wd["Wv"]}
        m.update(consts)
        in_maps.append(m)

    if "nc" not in _NC_CACHE:
        _NC_CACHE["nc"] = build_nc()
    res = run_bass_kernel_spmd(_NC_CACHE["nc"], in_maps,
                               core_ids=list(range(N_CORES)))
    outs = [r["out"] for r in res.results]
    return np.concatenate(outs, axis=0).reshape(B, L, H).astype(np.float32)



# revision 2
# speedup vs baseline: 1.3380x; 1.3380x over previous
"""Trainium2 Bass kernel for per-position multi-head "attention across heads".

Reference math (per position r):
    Q = x @ Wq.T ; K = x @ Wk.T ; V = x @ Wv.T          (H=1024, nh=16, hd=64)
    scores[r, i, j] = (1/8) * sum_d Q[r,i,d] * K[r,j,d]   -> [nh, nh] per position
    attn = softmax(scores, axis=-1)
    out[r, i, :] = sum_j attn[r,i,j] * V[r, j, :]

Strategy (8 NeuronCores, data-parallel over the 8192 = B*L positions; each
core handles R=1024 positions):
  - Inputs are cast to bf16 on the HOST (numpy) so the device reads x-shard
    [1024,1024] + Wq/Wk/Wv as bf16 and XBAR DMA-transpose loads produce
    x^T / W^T SBUF tiles directly from the input tensors (no on-device cast
    pass, no DRAM round trip).
  - Projections compute NATURAL-layout Q/K/V tiles [r, o] on the
    TensorEngine (PSUM fp32), evicted (DVE, cast bf16) to DRAM staging.
  - Q/K staging is [r, head, 128]: cols 0..63 = head data, cols 64..72 =
    constant "mask bias" rows (see below), cols 73..127 dead. One XBAR
    transpose per quarter yields position-major pm[d, r*16 + head]
    (partitions 0..72 live).
  - Scores for 8 positions at a time via ONE K=73 matmul:
      lhsT = K^T pm slice [73, 128], rhs = Q^T pm slice [73, 128]
      -> PSUM [(pos,j), (pos,i)].
    Contraction rows 64..72 implement the block-diagonal mask as math:
    rows 64+k (k<8) hold sqrt(C)*[pos==k] on both sides, row 72 holds
    +sqrt(C) (K side) and -sqrt(C) (Q side), so the matmul adds
    C*[pos_a==pos_b] - C to every score: off-diagonal (cross-position)
    garbage blocks get -C (C=12.625^2, scale*C ~ 19.9 => exp ~ 2e-9 ~ 0)
    while diagonal blocks are exactly unchanged. No mask multiply needed.
  - exp via ScalarE (no max subtraction: |scale*scores| <= ~4).
  - V staging is UNPADDED [r, head, 64]; the AV operand Vs[(r j), d] is a
    pure reshape of it, DMA-loaded (no transpose) as [128,(b),g,d] tiles
    with a 65th column of ones (softmax denominator trick).
  - AV via matmul with lhsT = E [(pos j), (pos i)], rhs = Vs -> PSUM
    [(pos,i), d|Z] is already in NATURAL row-major layout: normalize rows
    by 1/Z on the VectorE straight out of PSUM and DMA-scatter to out.
"""

import numpy as np

import concourse.bass as bass
import concourse.mybir as mybir
import concourse.tile as tile
from concourse import bacc

F32 = mybir.dt.float32
BF16 = mybir.dt.bfloat16

B, L, H = 4, 2048, 1024
NH, HD = 16, 64
P = 128
N_CORES = 8
R = (B * L) // N_CORES          # positions per core = 1024
KC = H // P                     # contraction chunks = 8
GS = 8                          # positions per attention group
GB = 4                          # groups per PSUM-bank batch
SCALE = 1.0 / np.sqrt(HD)
RTB = 12.625                    # sqrt(C); C=159.39, SCALE*C ~ 19.9
NBIAS = 9                       # 8 one-hot rows + 1 constant row


def build_nc(r_core=R):
    RC = r_core
    RT = RC // P                # x row tiles
    NGRP = RC // GS             # attention groups
    NBATCH = NGRP // GB         # group batches
    NQ = max(1, RT // 2)        # staging quarters (2 row tiles each)
    QR = RC // NQ               # rows per quarter

    nc = bacc.Bacc(None, target_bir_lowering=False, debug=False)

    xbf = nc.dram_tensor("xbf", [RC, H], BF16, kind="ExternalInput")
    Ws = {m: nc.dram_tensor(f"w{m}bf", [H, H], BF16, kind="ExternalInput")
          for m in "qkv"}
    bias_d = {m: nc.dram_tensor(f"bias_{m}", [P, NBIAS], BF16,
                                kind="ExternalInput") for m in "qk"}
    out = nc.dram_tensor("out", [RC, H], F32, kind="ExternalOutput")

    with tile.TileContext(nc) as tc:
        with tc.tile_pool(name="const", bufs=1) as constp, \
             tc.tile_pool(name="persist", bufs=1) as persist, \
             tc.tile_pool(name="dram", bufs=1, space="DRAM") as dram:
            bias_sb = {m: constp.tile([P, NBIAS], BF16, name=f"bias_sb_{m}")
                       for m in "qk"}
            for m in "qk":
                nc.sync.dma_start(bias_sb[m][:], bias_d[m][:])

            stag = {"q": dram.tile([RC, NH, P], BF16, name="stag_q"),
                    "k": dram.tile([RC, NH, P], BF16, name="stag_k"),
                    "v": dram.tile([RC, NH, HD], BF16, name="stag_v")}

            # persistent SBUF tensors
            xT = persist.tile([P, KC, RC], BF16)        # x^T chunks [h, kc, r]
            wT = {m: persist.tile([P, KC, H], BF16, name=f"wT_{m}")
                  for m in "qkv"}
            # position-major Q^T/K^T: pm[d, r*NH + head]; partitions 64..72
            # are the mask-bias rows, 73..127 dead.
            pm = {m: persist.tile([P, RC * NH], BF16, name=f"pm_{m}")
                  for m in "qk"}
            # Vs[(s j), b, g, d|1]: AV moving operand + ones column
            vs = persist.tile([P, NBATCH, GB, HD + 1], BF16, name="vs")
            nc.vector.memset(vs[:, :, :, HD], 1.0)

            # ---- phase 0: XBAR transpose loads of x^T and W^T ----
            for kc in range(KC):
                nc.sync.dma_start_transpose(
                    xT[:, kc, :], xbf[:, kc * P:(kc + 1) * P])
            for m in "qkv":
                for kc in range(KC):
                    nc.sync.dma_start_transpose(
                        wT[m][:, kc, :], Ws[m][:, kc * P:(kc + 1) * P])

            # ---- phase 1: projections -> staging -> pm / vs ----
            with tc.tile_pool(name="ev", bufs=1) as evp, \
                 tc.tile_pool(name="projps", bufs=4, space="PSUM") as projpsp:
                NBUF = 3
                qn_bufs = [evp.tile([P, NH, P], BF16, name=f"qn{i}")
                           for i in range(NBUF)]
                vn_bufs = [evp.tile([P, NH, HD], BF16, name=f"vn{i}")
                           for i in range(NBUF)]
                for t in qn_bufs:
                    nc.vector.memset(t[:, :, HD + NBIAS:P], 0.0)
                ti = 0
                for m in "qkv":
                    for rt in range(RT):
                        if m == "v":
                            tile_buf = vn_bufs[ti % NBUF]
                        else:
                            tile_buf = qn_bufs[ti % NBUF]
                            nc.vector.tensor_copy(
                                tile_buf[:, :, HD:HD + NBIAS],
                                bias_sb[m][:, None, :]
                                .to_broadcast((P, NH, NBIAS)))
                        ti += 1
                        for oh in range(2):
                            pp = projpsp.tile([P, 512], F32)
                            for kc in range(KC):
                                nc.tensor.matmul(
                                    pp[:],
                                    xT[:, kc, rt * P:(rt + 1) * P],
                                    wT[m][:, kc, oh * 512:(oh + 1) * 512],
                                    start=(kc == 0), stop=(kc == KC - 1))
                            nc.vector.tensor_copy(
                                tile_buf[:, oh * 8:(oh + 1) * 8, 0:HD],
                                pp[:].rearrange("p (i d) -> p i d", d=HD))
                        nc.gpsimd.dma_start(
                            stag[m][rt * P:(rt + 1) * P], tile_buf[:])
                        # per-quarter pm XBAR / vs reshape-load
                        if (rt + 1) % (RT // NQ if RT >= NQ else 1) == 0:
                            qf = (rt + 1) // (RT // NQ) - 1
                            rsl = slice(qf * QR, (qf + 1) * QR)
                            if m == "v":
                                bs = QR // (GB * GS)
                                nc.gpsimd.dma_start(
                                    vs[:, qf * bs:(qf + 1) * bs, :, 0:HD],
                                    stag["v"][rsl].rearrange(
                                        "(b g s) j d -> (s j) b g d",
                                        g=GB, s=GS))
                            else:
                                nc.sync.dma_start_transpose(
                                    pm[m][:, rsl.start * NH:rsl.stop * NH],
                                    stag[m][rsl]
                                    .rearrange("r i d -> (r i) d"))

            # ---- phase 2: attention ----
            KB = HD + NBIAS
            with tc.tile_pool(name="sps", bufs=2, space="PSUM") as spsp, \
                 tc.tile_pool(name="avps", bufs=2, space="PSUM") as avpsp, \
                 tc.tile_pool(name="att", bufs=3) as attp:
                for b in range(NBATCH):
                    ps = spsp.tile([P, GB, P], F32)
                    for g in range(GB):
                        c0 = (b * GB + g) * GS * NH
                        nc.tensor.matmul(
                            ps[:, g, :],
                            pm["k"][0:KB, c0:c0 + GS * NH],
                            pm["q"][0:KB, c0:c0 + GS * NH],
                            start=(g == 0), stop=(g == GB - 1))
                    E = attp.tile([P, GB, P], BF16, tag="E")
                    nc.scalar.activation(
                        E[:], ps[:], mybir.ActivationFunctionType.Exp,
                        scale=float(SCALE))
                    pav = avpsp.tile([P, GB, HD + 1], F32)
                    for g in range(GB):
                        nc.tensor.matmul(
                            pav[:, g, :], E[:, g, :], vs[:, b, g, :],
                            start=(g == 0), stop=(g == GB - 1))
                    rz = attp.tile([P, GB], F32, tag="rz")
                    nc.vector.reciprocal(rz[:], pav[:, :, HD])
                    onr = attp.tile([P, GB, HD], F32, tag="onr")
                    nc.vector.tensor_tensor(
                        onr[:], pav[:, :, 0:HD],
                        rz[:, :, None].to_broadcast((P, GB, HD)),
                        mybir.AluOpType.mult)
                    nc.gpsimd.dma_start(
                        out[b * GB * GS:(b + 1) * GB * GS, :]
                        .rearrange("(g s) (i d) -> (s i) g d", s=GS, d=HD),
                        onr[:])

    nc.compile()
    return nc


def _consts():
    import ml_dtypes
    bq = np.zeros((P, NBIAS), np.float32)
    bk = np.zeros((P, NBIAS), np.float32)
    for p in range(P):
        bq[p, p % GS] = RTB
        bk[p, p % GS] = RTB
    bq[:, GS] = -RTB
    bk[:, GS] = RTB
    return {"bias_q": bq.astype(ml_dtypes.bfloat16),
            "bias_k": bk.astype(ml_dtypes.bfloat16)}


_NC_CACHE = {}


def make_in_maps(x, Wq, Wk, Wv):
    import ml_dtypes
    bf = ml_dtypes.bfloat16
    xf = np.asarray(x, np.float32).reshape(B * L, H).astype(bf)
    wbf = {m: np.asarray(w, np.float32).astype(bf)
           for m, w in (("q", Wq), ("k", Wk), ("v", Wv))}
    consts = _consts()
    maps = []
    for c in range(N_CORES):
        m = {"xbf": np.ascontiguousarray(xf[c * R:(c + 1) * R]),
             "wqbf": wbf["q"], "wkbf": wbf["k"], "wvbf": wbf["v"]}
        m.update(consts)
        maps.append(m)
    return maps


def kernel(x, Wq, Wk, Wv):
    from concourse.bass_utils import run_bass_kernel_spmd

    if "nc" not in _NC_CACHE:
        _NC_CACHE["nc"] = build_nc()
    res = run_bass_kernel_spmd(_NC_CACHE["nc"], make_in_maps(x, Wq, Wk, Wv),
                               core_ids=list(range(N_CORES)))
    outs = [r["out"] for r in res.results]
    return np.concatenate(outs, axis=0).reshape(B, L, H).astype(np.float32)


# revision 3
# speedup vs baseline: 1.8515x; 1.3838x over previous
"""Trainium2 Bass kernel for per-position multi-head "attention across heads".

Reference math (per position r):
    Q = x @ Wq.T ; K = x @ Wk.T ; V = x @ Wv.T          (H=1024, nh=16, hd=64)
    scores[r, i, j] = (1/8) * sum_d Q[r,i,d] * K[r,j,d]   -> [nh, nh] per position
    attn = softmax(scores, axis=-1)
    out[r, i, :] = sum_j attn[r,i,j] * V[r, j, :]

Strategy (8 NeuronCores, data-parallel over the 8192 = B*L positions; each
core handles R=1024 positions):
  - x^T shard [1024,1024] and Wq^T/Wk^T/Wv^T are pre-transposed AND cast to
    bf16 on the HOST (numpy), so the device just does plain contiguous DMA
    loads of the projection operands (contraction dim on partitions) - no
    on-device casts, no input-side XBAR transposes.
  - Projections compute NATURAL-layout Q/K/V tiles [r, o] on the
    TensorEngine (PSUM fp32), evicted (DVE, cast bf16) to DRAM staging.
  - Q/K staging is [r, head, 128]: cols 0..63 = head data, cols 64..72 =
    constant "mask bias" rows (see below), cols 73..127 dead. One XBAR
    transpose per quarter yields position-major pm[d, r*16 + head]
    (partitions 0..72 live).
  - Scores for 8 positions at a time via ONE K=73 matmul:
      lhsT = K^T pm slice [73, 128], rhs = Q^T pm slice [73, 128]
      -> PSUM [(pos,j), (pos,i)].
    Contraction rows 64..72 implement the block-diagonal mask as math:
    rows 64+k (k<8) hold sqrt(C)*[pos==k] on both sides, row 72 holds
    +sqrt(C) (K side) and -sqrt(C) (Q side), so the matmul adds
    C*[pos_a==pos_b] - C to every score: off-diagonal (cross-position)
    garbage blocks get -C (C=12.625^2, scale*C ~ 19.9 => exp ~ 2e-9 ~ 0)
    while diagonal blocks are exactly unchanged. No mask multiply needed.
  - exp via ScalarE (no max subtraction: |scale*scores| <= ~4) into a
    persistent E_all buffer; ALL score matmuls run before ALL AV matmuls so
    the PE stream is dense (no per-batch PE<->ACT latency coupling).
  - V staging is UNPADDED [r, head, 64]; the AV operand Vs[(r j), d] is a
    pure reshape of it, DMA-loaded (no transpose) as [128,(b),g,d] tiles
    with a 65th column of ones (softmax denominator trick).
  - AV via matmul with lhsT = E [(pos j), (pos i)], rhs = Vs -> PSUM
    [(pos,i), d|Z] is already in NATURAL row-major layout: normalize rows
    by 1/Z on the VectorE straight out of PSUM and DMA-scatter to out.
"""

import numpy as np

import concourse.bass as bass
import concourse.mybir as mybir
import concourse.tile as tile
from concourse import bacc

F32 = mybir.dt.float32
BF16 = mybir.dt.bfloat16

B, L, H = 4, 2048, 1024
NH, HD = 16, 64
P = 128
N_CORES = 8
R = (B * L) // N_CORES          # positions per core = 1024
KC = H // P                     # contraction chunks = 8
GS = 8                          # positions per attention group
GB = 4                          # groups per PSUM-bank batch
SCALE = 1.0 / np.sqrt(HD)
RTB = 12.625                    # sqrt(C); C=159.39, SCALE*C ~ 19.9
NBIAS = 9                       # 8 one-hot rows + 1 constant row


def build_nc(r_core=R):
    RC = r_core
    RT = RC // P                # x row tiles
    NGRP = RC // GS             # attention groups
    NBATCH = NGRP // GB         # group batches
    NQ = max(1, RT // 2)        # staging quarters (2 row tiles each)
    QR = RC // NQ               # rows per quarter

    nc = bacc.Bacc(None, target_bir_lowering=False, debug=False)

    xT_d = nc.dram_tensor("xT", [H, RC], BF16, kind="ExternalInput")
    wT_d = {m: nc.dram_tensor(f"wT_{m}", [H, H], BF16, kind="ExternalInput")
            for m in "qkv"}
    bias_d = {m: nc.dram_tensor(f"bias_{m}", [P, NBIAS], BF16,
                                kind="ExternalInput") for m in "qk"}
    out = nc.dram_tensor("out", [RC, H], F32, kind="ExternalOutput")

    with tile.TileContext(nc) as tc:
        with tc.tile_pool(name="const", bufs=1) as constp, \
             tc.tile_pool(name="persist", bufs=1) as persist, \
             tc.tile_pool(name="dram", bufs=1, space="DRAM") as dram:
            bias_sb = {m: constp.tile([P, NBIAS], BF16, name=f"bias_sb_{m}")
                       for m in "qk"}
            for m in "qk":
                nc.sync.dma_start(bias_sb[m][:], bias_d[m][:])

            stag = {"q": dram.tile([RC, NH, P], BF16, name="stag_q"),
                    "k": dram.tile([RC, NH, P], BF16, name="stag_k"),
                    "v": dram.tile([RC, NH, HD], BF16, name="stag_v")}

            # persistent SBUF tensors
            xT = persist.tile([P, KC, RC], BF16)        # x^T chunks [h, kc, r]
            # position-major Q^T/K^T: pm[d, r*NH + head]; partitions 64..72
            # are the mask-bias rows, 73..127 dead.
            pm = {m: persist.tile([P, RC * NH], BF16, name=f"pm_{m}")
                  for m in "qk"}
            # Vs[(s j), b, g, d|1]: AV moving operand + ones column
            vs = persist.tile([P, NBATCH, GB, HD + 1], BF16, name="vs")
            nc.vector.memset(vs[:, :, :, HD], 1.0)

            # ---- phase 0+1: input loads, projections -> staging -> pm/vs ----
            with tc.tile_pool(name="wt", bufs=1) as wtp, \
                 tc.tile_pool(name="ev", bufs=1) as evp, \
                 tc.tile_pool(name="projps", bufs=4, space="PSUM") as projpsp:
                wT = {m: wtp.tile([P, KC, H], BF16, name=f"wT_{m}")
                      for m in "qkv"}
                # plain contiguous loads of pre-transposed operands
                # (scalar HWDGE ring; sync ring is reserved for pm XBARs)
                nc.scalar.dma_start(
                    xT[:], xT_d.rearrange("(kc p) r -> p kc r", p=P))
                for m in "qkv":
                    nc.scalar.dma_start(
                        wT[m][:], wT_d[m].rearrange("(kc p) o -> p kc o", p=P))

                NBUF = 3
                qn_bufs = [evp.tile([P, NH, P], BF16, name=f"qn{i}")
                           for i in range(NBUF)]
                vn_bufs = [evp.tile([P, NH, HD], BF16, name=f"vn{i}")
                           for i in range(NBUF)]
                for t in qn_bufs:
                    nc.vector.memset(t[:, :, HD + NBIAS:P], 0.0)
                ti = 0
                for m in "qkv":
                    for rt in range(RT):
                        if m == "v":
                            tile_buf = vn_bufs[ti % NBUF]
                        else:
                            tile_buf = qn_bufs[ti % NBUF]
                            nc.vector.tensor_copy(
                                tile_buf[:, :, HD:HD + NBIAS],
                                bias_sb[m][:, None, :]
                                .to_broadcast((P, NH, NBIAS)))
                        ti += 1
                        for oh in range(2):
                            pp = projpsp.tile([P, 512], F32)
                            for kc in range(KC):
                                nc.tensor.matmul(
                                    pp[:],
                                    xT[:, kc, rt * P:(rt + 1) * P],
                                    wT[m][:, kc, oh * 512:(oh + 1) * 512],
                                    start=(kc == 0), stop=(kc == KC - 1))
                            nc.vector.tensor_copy(
                                tile_buf[:, oh * 8:(oh + 1) * 8, 0:HD],
                                pp[:].rearrange("p (i d) -> p i d", d=HD))
                        nc.gpsimd.dma_start(
                            stag[m][rt * P:(rt + 1) * P], tile_buf[:])
                        # per-quarter pm XBAR / vs reshape-load
                        if (rt + 1) % max(1, RT // NQ) == 0:
                            qf = (rt + 1) // max(1, RT // NQ) - 1
                            rsl = slice(qf * QR, (qf + 1) * QR)
                            if m == "v":
                                bs = QR // (GB * GS)
                                nc.gpsimd.dma_start(
                                    vs[:, qf * bs:(qf + 1) * bs, :, 0:HD],
                                    stag["v"][rsl].rearrange(
                                        "(b g s) j d -> (s j) b g d",
                                        g=GB, s=GS))
                            else:
                                nc.sync.dma_start_transpose(
                                    pm[m][:, rsl.start * NH:rsl.stop * NH],
                                    stag[m][rsl]
                                    .rearrange("r i d -> (r i) d"))

            # ---- phase 2: attention (all scores+exp, then all AV) ----
            KB = HD + NBIAS
            with tc.tile_pool(name="eall", bufs=1) as eallp, \
                 tc.tile_pool(name="sps", bufs=3, space="PSUM") as spsp, \
                 tc.tile_pool(name="avps", bufs=4, space="PSUM") as avpsp, \
                 tc.tile_pool(name="att", bufs=4) as attp:
                E_all = eallp.tile([P, NBATCH, GB, P], BF16, name="E_all")
                for b in range(NBATCH):
                    ps = spsp.tile([P, GB, P], F32)
                    for g in range(GB):
                        c0 = (b * GB + g) * GS * NH
                        nc.tensor.matmul(
                            ps[:, g, :],
                            pm["k"][0:KB, c0:c0 + GS * NH],
                            pm["q"][0:KB, c0:c0 + GS * NH],
                            start=(g == 0), stop=(g == GB - 1))
                    nc.scalar.activation(
                        E_all[:, b], ps[:], mybir.ActivationFunctionType.Exp,
                        scale=float(SCALE))
                for b in range(NBATCH):
                    pav = avpsp.tile([P, GB, HD + 1], F32)
                    for g in range(GB):
                        nc.tensor.matmul(
                            pav[:, g, :], E_all[:, b, g, :], vs[:, b, g, :],
                            start=(g == 0), stop=(g == GB - 1))
                    rz = attp.tile([P, GB], F32, tag="rz")
                    nc.vector.reciprocal(rz[:], pav[:, :, HD])
                    onr = attp.tile([P, GB, HD], F32, tag="onr")
                    nc.vector.tensor_tensor(
                        onr[:], pav[:, :, 0:HD],
                        rz[:, :, None].to_broadcast((P, GB, HD)),
                        mybir.AluOpType.mult)
                    nc.sync.dma_start(
                        out[b * GB * GS:(b + 1) * GB * GS, :]
                        .rearrange("(g s) (i d) -> (s i) g d", s=GS, d=HD),
                        onr[:])

    nc.compile()
    return nc


def _consts():
    import ml_dtypes
    bq = np.zeros((P, NBIAS), np.float32)
    bk = np.zeros((P, NBIAS), np.float32)
    for p in range(P):
        bq[p, p % GS] = RTB
        bk[p, p % GS] = RTB
    bq[:, GS] = -RTB
    bk[:, GS] = RTB
    return {"bias_q": bq.astype(ml_dtypes.bfloat16),
            "bias_k": bk.astype(ml_dtypes.bfloat16)}


_NC_CACHE = {}


def make_in_maps(x, Wq, Wk, Wv):
    import ml_dtypes
    bf = ml_dtypes.bfloat16
    xTf = np.asarray(x, np.float32).reshape(B * L, H).astype(bf).T
    wT = {m: np.ascontiguousarray(np.asarray(w, np.float32).astype(bf).T)
          for m, w in (("q", Wq), ("k", Wk), ("v", Wv))}
    consts = _consts()
    maps = []
    for c in range(N_CORES):
        m = {"xT": np.ascontiguousarray(xTf[:, c * R:(c + 1) * R]),
             "wT_q": wT["q"], "wT_k": wT["k"], "wT_v": wT["v"]}
        m.update(consts)
        maps.append(m)
    return maps


def kernel(x, Wq, Wk, Wv):
    from concourse.bass_utils import run_bass_kernel_spmd

    if "nc" not in _NC_CACHE:
        _NC_CACHE["nc"] = build_nc()
    res = run_bass_kernel_spmd(_NC_CACHE["nc"], make_in_maps(x, Wq, Wk, Wv),
                               core_ids=list(range(N_CORES)))
    outs = [r["out"] for r in res.results]
    return np.concatenate(outs, axis=0).reshape(B, L, H).astype(np.float32)
